# revision 38
# baseline (speedup 1.0000x reference)
"""GAT-style GNN message passing on 8 TRN2 NeuronCores — collective-free.

Math: with LEAK=1 the leaky-relu is identity, so
  e[i,j,h] = e_src[i,h] + e_dst[j,h]
and softmax over j cancels e_src (and any row max) exactly:
  attn[i,j,h] = adj[i,j]*exp(e_dst[j,h]) / sum_j adj[i,j]*exp(e_dst[j,h])
  out[i,(h,f)] = (adj @ (z*h))[i,(h,f)] / (adj @ z)[i,h],  z = exp(e_dst)
then elu + log_softmax per row. log_softmax is shift invariant, so
elu(x) is computed as relu(x) + exp(min(x,0)) (drops the uniform -1),
and no max subtraction is needed (y is bounded in (0, ~10]).

Sharding: rows (query nodes) of adj/out across 8 cores. The h = x@W
computation (cheap: 0.6 GFLOP) is REPLICATED on every core from a full
copy of x: zero collectives (an AllGather version paid a ~40us CC
barrier), zero cross-core dependencies, PE stays HAM-warm.

Dtypes (runs are HBM-bound, so bytes are everything): x fp8e4 (4MB
replicated), W bf16 (mixed bf16-stationary x fp8-moving matmuls are
legal on the PE, HW-verified), adjacency fp8e4 (0/1 exact, 2MB/core),
G=[z*h | z] bf16 stationary vs the fp8 moving adjT. fp32 PSUM accum.

Pipeline: x streams in GROUP-MAJOR layout — each 512KB transfer holds
ALL EIGHT k-chunks for one 512-node group, so the full chain
  h-matmuls -> evac -> 4x(transpose, z=exp, G=z*h) -> 4 agg matmuls
completes per group and runs concurrently with the next group's DMA.
Adjacency splits are interleaved (x0 a0 x1 x2 a1 x3 x4 a2 x5 x6 a3 x7)
so aggregation never waits on adj and the tail after the last byte is
just one group's chain + the postprocess.

Per-core device program (R = N/8 = 512 rows, P = 128, NG = 8 groups):
  inputs:  xt [128, NG*KC*512] fp8  xt[p, g*4096+kc*512+n] = x[g*512+n, kc*128+p]
           wt [128, KC*72] bf16     wt[p, kc*72+e] = w_ext[kc*128+p, e]
                                    (w_ext = [W | W @ blockdiag-reduced a_dst])
           at [128, NJ*R]  fp8      at[p, nj*R+r]  = adj[c*R+r, nj*128+p]
  output:  out_p [128, RC*64] f32   out_p[p, q*64+f] = out[c*R+q*128+p, (h,f)]
"""

import sys

import numpy as np

if "/opt/trn_rl_repo" not in sys.path:
    sys.path.insert(0, "/opt/trn_rl_repo")

import ml_dtypes  # noqa: E402

import concourse.bass as bass  # noqa: E402
import concourse.tile as tile  # noqa: E402
from concourse import bacc, mybir  # noqa: E402
from concourse.bass_utils import run_bass_kernel_spmd  # noqa: E402
from concourse.masks import make_identity  # noqa: E402

N_CORES = 8
H = 8
F = 8
HF = H * F  # 64
EXT = HF + H  # 72: [g | z]
K_IN = 1024
P = 128

FP32 = mybir.dt.float32
BF16 = mybir.dt.bfloat16
FP8 = mybir.dt.float8e4
AFT = mybir.ActivationFunctionType
ALU = mybir.AluOpType


def _bcast_head(ap_ph):
    """[P, H] AP -> [P, H, F] AP broadcasting each head value over F."""
    return bass.AP(
        tensor=ap_ph.tensor,
        offset=ap_ph.offset,
        ap=[ap_ph.ap[0], ap_ph.ap[1], [0, F]],
    )


def build_bass(n_nodes: int) -> bass.Bass:
    R = n_nodes // N_CORES
    KC = K_IN // P  # k-chunks for the h matmul
    NJ = n_nodes // P  # j (neighbor) 128-chunks
    NG = n_nodes // 512  # 512-node groups
    RC = R // P  # 128-row output chunks per core
    assert R % P == 0

    # Bacc finalize() runs move_matmul_waits_to_ldweights +
    # generate_event_semaphores, which legalize multi-wait instructions
    # for walrus (TRN2 allows at most 1 sync wait per instruction).
    nc = bacc.Bacc(num_devices=N_CORES)

    xt = nc.declare_dram_parameter("xt", [P, NG * KC * 512], FP8, isOutput=False)
    wt = nc.declare_dram_parameter("wt", [P, KC * EXT], BF16, isOutput=False)
    at = nc.declare_dram_parameter("at", [P, NJ * R], FP8, isOutput=False)
    out = nc.declare_dram_parameter("out", [P, RC * HF], FP32, isOutput=True)

    with tile.TileContext(nc) as tc:
        with (
            tc.tile_pool(name="singles", bufs=1) as singles,
            tc.tile_pool(name="xstream", bufs=5) as xstream,
            tc.tile_pool(name="hbuf", bufs=4) as hbuf,
            tc.tile_pool(name="hpsum", bufs=2, space="PSUM") as hpsum,
            tc.tile_pool(name="outpsum", bufs=1, space="PSUM") as outpsum,
            tc.tile_pool(name="smallpsum", bufs=3, space="PSUM") as smallpsum,
            tc.tile_pool(name="work", bufs=6) as work,
            tc.tile_pool(name="post", bufs=4) as post,
        ):
            ident = singles.tile([P, P], FP32)
            make_identity(nc, ident)

            # --- loads (p-major, one contiguous run per partition) ---
            w_sb = singles.tile([P, KC, EXT], BF16)
            nc.sync.dma_start(
                out=w_sb, in_=wt[:].rearrange("p (c e) -> p c e", c=KC)
            )
            xt_view = xt[:].rearrange("p (g c n) -> p g c n", g=NG, c=KC)
            at_sb = singles.tile([P, NJ, R], FP8)
            at_view = at[:].rearrange("p (n r) -> p n r", n=NJ)

            g_sb = singles.tile([P, NJ, EXT], BF16)
            outT_ps = outpsum.tile([EXT, R], FP32)

            # Streamed per-group pipeline. DMA order on the sync queue:
            # w x0 a0 x1 x2 a1 x3 x4 a2 x5 x6 a3 x7 out
            for g in range(NG):
                xbuf = xstream.tile([P, KC, 512], FP8, tag="xs")
                nc.sync.dma_start(out=xbuf, in_=xt_view[:, g])
                if g % 2 == 0:
                    # Each adjacency split MUST be emitted before the
                    # aggregation matmuls of the two groups it feeds:
                    # Tile's dependency tracking is trace-ordered, so a
                    # consumer emitted before its producer silently
                    # reads stale memory (and races the later DMA).
                    s = g // 2
                    nc.sync.dma_start(
                        out=at_sb[:, 8 * s : 8 * s + 8, :],
                        in_=at_view[:, 8 * s : 8 * s + 8, :],
                    )
                hg_ps = hpsum.tile([EXT, 512], FP32, tag="hps")
                for kc in range(KC):
                    nc.tensor.matmul(
                        hg_ps,
                        lhsT=w_sb[:, kc, :],
                        rhs=xbuf[:, kc, :],
                        start=(kc == 0),
                        stop=(kc == KC - 1),
                    )
                hb = hbuf.tile([EXT, 512], FP32, tag="hb")
                nc.vector.tensor_copy(hb, hg_ps)

                # All 4 chunk transposes land in ONE psum tile, then z/g is
                # computed with ONE exp / ONE broadcast-multiply / ONE copy
                # for the whole group (7 engine ops instead of 16 — the
                # chunk pipeline was cross-engine-sync bound).
                h_ps4 = smallpsum.tile([P, 4, EXT], FP32, tag="smallps")
                for q2 in range(4):
                    nc.tensor.transpose(
                        h_ps4[:, q2, :],
                        hb[:, q2 * P : (q2 + 1) * P],
                        ident[:EXT, :EXT],
                    )
                z4 = work.tile([P, 4, H], FP32, tag="z")
                nc.scalar.activation(z4, h_ps4[:, :, HF:EXT], AFT.Exp)
                z4b = bass.AP(
                    tensor=z4.tensor,
                    offset=z4.offset,
                    ap=[z4.ap[0], z4.ap[1], z4.ap[2], [0, F]],
                )
                nc.vector.tensor_mul(
                    g_sb[:, 4 * g : 4 * g + 4, 0:HF].rearrange(
                        "p c (h f) -> p c h f", h=H
                    ),
                    h_ps4[:, :, 0:HF].rearrange("p c (h f) -> p c h f", h=H),
                    z4b,
                )
                nc.vector.tensor_copy(g_sb[:, 4 * g : 4 * g + 4, HF:EXT], z4)

                # aggregate this group's j-chunks: outT += G_n.T @ adjT_n
                for n2 in range(4):
                    n = 4 * g + n2
                    nc.tensor.matmul(
                        outT_ps,
                        lhsT=g_sb[:, n, :],
                        rhs=at_sb[:, n, :],
                        start=(n == 0),
                        stop=(n == NJ - 1),
                    )

            outT_sb = singles.tile([EXT, R], FP32)
            nc.vector.tensor_copy(outT_sb, outT_ps)

            # --- postprocess, batched per stage across the RC chunks ---
            o_ps = [None] * RC
            for q in range(RC):
                o_ps[q] = smallpsum.tile([P, EXT], FP32, tag="smallps", name=f"o{q}")
                nc.tensor.transpose(
                    o_ps[q], outT_sb[:, q * P : (q + 1) * P], ident[:EXT, :EXT]
                )
            xo = [None] * RC
            for q in range(RC):
                rd = work.tile([P, H], FP32, tag="rd")
                nc.vector.reciprocal(rd, o_ps[q][:, HF:EXT])
                xo[q] = post.tile([P, HF], FP32, tag="xo", name=f"xo{q}")
                nc.vector.tensor_mul(
                    xo[q].rearrange("p (h f) -> p h f", h=H),
                    o_ps[q][:, 0:HF].rearrange("p (h f) -> p h f", h=H),
                    _bcast_head(rd),
                )
            # y = relu(xo) + exp(min(xo, 0))  (= elu + 1; log_softmax shift-safe)
            yo = [None] * RC
            eo = [None] * RC
            for q in range(RC):
                mo = work.tile([P, HF], FP32, tag="mo")
                nc.vector.tensor_scalar_min(mo, xo[q], 0.0)
                eo[q] = post.tile([P, HF], FP32, tag="eo", name=f"eo{q}")
                nc.scalar.activation(eo[q], mo, AFT.Exp)
            for q in range(RC):
                yo[q] = post.tile([P, HF], FP32, tag="yo", name=f"yo{q}")
                nc.vector.scalar_tensor_tensor(
                    out=yo[q], in0=xo[q], scalar=0.0, in1=eo[q],
                    op0=ALU.max, op1=ALU.add,
                )
            # log-softmax over the 64 features (no max subtraction needed:
            # y in (0, ~10], exp stays in fp32 range); batch Exp then Ln to
            # avoid ACT table-set thrash.
            ex = [None] * RC
            sm = [None] * RC
            for q in range(RC):
                ex[q] = post.tile([P, HF], FP32, tag="ex", name=f"ex{q}")
                nc.scalar.activation(ex[q], yo[q], AFT.Exp)
            for q in range(RC):
                sm[q] = post.tile([P, 1], FP32, tag="sm", name=f"sm{q}")
                nc.vector.reduce_sum(sm[q], ex[q], axis=mybir.AxisListType.X)
            out_sb = singles.tile([P, RC, HF], FP32)
            out_view = out[:].rearrange("p (q e) -> p q e", q=RC)
            for q in range(RC):
                ls = work.tile([P, 1], FP32, tag="ls")
                nc.scalar.activation(ls, sm[q], AFT.Ln)
                nc.vector.tensor_scalar_sub(out_sb[:, q, :], yo[q], ls)
                if q % 2 == 1:
                    # store each half as soon as it's ready (512B/partition
                    # runs keep DMA at line rate); the final HBM write
                    # receipt (~2us) is on the critical path, so the last
                    # bytes should leave as early as possible.
                    nc.sync.dma_start(
                        out=out_view[:, q - 1 : q + 1, :],
                        in_=out_sb[:, q - 1 : q + 1, :],
                    )

    # Force all ACT activations (Exp + Ln) onto the one table set containing
    # both, so only ONE ACT_TABLE_LOAD is emitted (early, hidden under DMA)
    # instead of a ~1.3us reload at every Exp<->Ln switch. Set indices must
    # stay aligned with act_info.json, so empty the other sets rather than
    # filtering the list.
    orig_gat = bacc.get_activation_tables

    def _one_set(arch):
        return {
            k: (v if k == "natural_log_exp_and_others" else set())
            for k, v in orig_gat(arch).items()
        }

    bacc.get_activation_tables = _one_set
    try:
        nc.finalize()
    finally:
        bacc.get_activation_tables = orig_gat
    return nc


def _host_prep(x, adj, W, a_dst, n_nodes):
    """Build per-core input maps."""
    R = n_nodes // N_CORES
    NJ = n_nodes // P
    KC = K_IN // P
    Wd = np.einsum(
        "khf,hf->kh", W.reshape(K_IN, H, F), a_dst, dtype=np.float32
    ).astype(np.float32)
    w_ext = np.concatenate([W, Wd], axis=1)  # [1024, 72] fp32
    # wt[p, kc*72+e] = w_ext[kc*128+p, e]
    wt = np.ascontiguousarray(
        w_ext.reshape(KC, P, EXT).transpose(1, 0, 2).reshape(P, KC * EXT)
    ).astype(ml_dtypes.bfloat16)
    # xt[p, g*4096+kc*512+n2] = x[g*512+n2, kc*128+p]  (shared by all cores)
    xT = x.astype(ml_dtypes.float8_e4m3).T  # [1024, 4096]
    xt = np.ascontiguousarray(
        xT.reshape(KC, P, n_nodes // 512, 512).transpose(1, 2, 0, 3)
        .reshape(P, n_nodes * KC)
    )
    # adj as fp8e4: 0/1 exact (1.0 == 0x38); byte-level build beats .astype.
    adj_u8 = (adj.astype(np.uint8) * np.uint8(0x38)).view(ml_dtypes.float8_e4m3)
    in_maps = []
    for c in range(N_CORES):
        rows = slice(c * R, (c + 1) * R)
        # at[p, nj*R+r] = adj[c*R+r, nj*128+p]
        at = np.ascontiguousarray(
            adj_u8[rows].T.reshape(NJ, P, R).transpose(1, 0, 2).reshape(P, NJ * R)
        )
        in_maps.append({"xt": xt, "at": at, "wt": wt})
    return in_maps


_BUILT = {}


def run(x, adj, W, a_dst, trace=False):
    n_nodes = x.shape[0]
    R = n_nodes // N_CORES
    RC = R // P
    if n_nodes not in _BUILT:
        _BUILT[n_nodes] = build_bass(n_nodes)
    nc = _BUILT[n_nodes]
    in_maps = _host_prep(x, adj, W, a_dst, n_nodes)
    res = run_bass_kernel_spmd(
        nc, in_maps, list(range(N_CORES)), trace=trace
    )
    blocks = []
    for c in range(N_CORES):
        o = res.results[c]["out"]  # [P, RC*HF] p-major
        blocks.append(
            o.reshape(P, RC, HF).transpose(1, 0, 2).reshape(R, HF)
        )
    return np.concatenate(blocks, axis=0).astype(np.float32), res


def kernel(x, adj, W, a_src, a_dst):
    x = np.asarray(x, dtype=np.float32)
    adj = np.asarray(adj)
    W = np.asarray(W, dtype=np.float32)
    a_dst = np.asarray(a_dst, dtype=np.float32)
    out, _ = run(x, adj, W, a_dst, trace=False)
    return out
